# revision 20
# baseline (speedup 1.0000x reference)
"""CoAtNet relative attention kernel for Trainium2 (Bass/Tile), 8 NeuronCores.

Problem (per full input):
  x [16, 256, 32, 32] f32; Wq/Wk/Wv [256, 256]; Wo [256, 256]; bo [256];
  rel_bias [8, 3969]; rel_idx [1024, 1024] int32 (static pattern).
  out[b] = softmax(q k^T / sqrt(d) + bias) v  projected back, heads=8, d=32.

Sharding: data-parallel over batch — each of the 8 cores handles 2 batches
with identical programs (SPMD, no collectives).

Key structural facts used:
  * rel_idx[p, q] == (q - p) + 1056 exactly (the reference's quirky *W stride
    collapses the 2D relative index to 1D Toeplitz).  So the [1024, 1024]
    bias matrix per head is bias[p, q] = rel_bias[h, q - p + 1056] and any
    [128, width] tile of it (keys on partitions) is a contiguous slice of a
    small "sheared" tile  G[h, i, j'] = rel_bias[h, 1952 + i - j']  of shape
    [128, 1920].  No gather on device at all.  The bias is applied as
    exp(S+B) = exp(S) * exp(B) with exp(B) precomputed, so the application
    is a bf16 2x-mode multiply instead of an fp32 1x add.
  * Everything is computed in "transposed" layout so no transposes are ever
    needed: x arrives as [c, n] per batch; Q^T/K^T = W @ x are [d_all, n];
    scores are built as S^T [keys, queries]; P@V uses lhsT = [V | 1]
    directly (the ones column emits the softmax denominator as a 33rd
    output row, one accumulation group per PSUM bank); and the final
    projection produces out^T [c, n], exactly the output memory layout.
  * Stage B processes 4 heads at once with row-tiled (tile_position)
    concurrent K=32 matmuls so the PE array stays dense/warm, and exp runs
    as one 2048-wide ACTIVATE per strip.
"""

import numpy as np
from contextlib import ExitStack

import concourse.bass as bass
import concourse.bacc as bacc
import concourse.mybir as mybir
import concourse.tile as tile
from concourse import bass_utils
from concourse._compat import with_exitstack

HEADS = 8
D = 32  # head dim
C = 256  # channels = heads * D
N = 1024  # tokens = 32 * 32
B_LOC = 2  # batches per core
N_CORES = 8
SCALE = D ** -0.5
GW = 1920  # sheared bias tile width
G0 = 1952  # G[h, i, j'] = rel_bias[h, G0 + i - j']

F32 = mybir.dt.float32
BF16 = mybir.dt.bfloat16
AF = mybir.ActivationFunctionType


@with_exitstack
def _emit(ctx: ExitStack, tc: tile.TileContext, io: dict):
    nc = tc.nc
    x, wqT, wkT, wvT, woT, bo, eb, out = (
        io[k] for k in ("x", "wqT", "wkT", "wvT", "woT", "bo", "eb", "out")
    )

    persist = ctx.enter_context(tc.tile_pool(name="persist", bufs=1))
    stexp_pool = ctx.enter_context(tc.tile_pool(name="stexp", bufs=3))
    small = ctx.enter_context(tc.tile_pool(name="small", bufs=2))
    outp = ctx.enter_context(tc.tile_pool(name="outp", bufs=4))
    dram_pool = ctx.enter_context(tc.tile_pool(name="dram", bufs=2, space="DRAM"))
    # PSUM: st [128, 2048] (4 banks) + otden [33, 2048] (4 banks) = all 8.
    # The projection stages ping-pong between the two pools for overlap.
    ps_st = ctx.enter_context(tc.tile_pool(name="ps_st", bufs=1, space="PSUM"))
    ps_ot = ctx.enter_context(tc.tile_pool(name="ps_ot", bufs=1, space="PSUM"))

    def proj_psum(i):
        if i % 2 == 0:
            return ps_st.tile([128, 512], F32, tag="st", name="st_ps")
        return ps_ot.tile([128, 512], F32, tag="otden", name="otden_ps")

    # ---------- constants / weights / inputs ----------
    wq_sb, wk_sb, wv_sb, wo_sb = [], [], [], []
    for cc in range(2):
        for lst, src, nm in (
            (wq_sb, wqT, "wq"),
            (wk_sb, wkT, "wk"),
            (wv_sb, wvT, "wv"),
            (wo_sb, woT, "wo"),
        ):
            t = persist.tile([128, C], BF16, tag=f"{nm}{cc}", name=f"{nm}{cc}")
            nc.sync.dma_start(out=t[:], in_=src[128 * cc : 128 * (cc + 1), :])
            lst.append(t)
    bo_sb = []
    for cc in range(2):
        t = persist.tile([128, 1], F32, tag=f"bo{cc}", name=f"bo{cc}")
        nc.sync.dma_start(out=t[:], in_=bo[128 * cc : 128 * (cc + 1), :])
        bo_sb.append(t)
    # all 8 heads' exp-of-bias sheared tiles, side by side
    eb_sb = persist.tile([128, HEADS * GW], BF16, tag="eb", name="eb_sb")
    for h in range(HEADS):
        nc.sync.dma_start(out=eb_sb[:, GW * h : GW * (h + 1)], in_=eb[h])

    x_sb = [[persist.tile([128, N], BF16, tag=f"x{b}_{cc}", name=f"x{b}_{cc}") for cc in range(2)] for b in range(B_LOC)]
    for b in range(B_LOC):
        for cc in range(2):
            nc.sync.dma_start(out=x_sb[b][cc][:], in_=x[b, 128 * cc : 128 * (cc + 1), :])

    # ---------- stage A: projections ----------
    # qT/kT: [o, n] (o = h*32 + d), computed as (W^T)^T @ x = W @ x.
    qT_sb = [[persist.tile([128, N], BF16, tag=f"qT{b}_{oc}", name=f"qT{b}_{oc}") for oc in range(2)] for b in range(B_LOC)]
    kT_sb = [[persist.tile([128, N], BF16, tag=f"kT{b}_{oc}", name=f"kT{b}_{oc}") for oc in range(2)] for b in range(B_LOC)]
    # v: natural [n, o] layout, 8 row tiles of 128 tokens, with a ones column
    # appended per head (33 cols/head); the P@V matmul then emits the softmax
    # denominator as a 33rd output row in the same accumulation group.
    v_sb = [[persist.tile([128, 33 * HEADS], BF16, tag=f"v{b}_{nt}", name=f"v{b}_{nt}") for nt in range(8)] for b in range(B_LOC)]

    pi = 0
    for b in range(B_LOC):
        for oc in range(2):
            for nc2 in range(2):
                for w_sb, dst in ((wq_sb, qT_sb), (wk_sb, kT_sb)):
                    pq = proj_psum(pi)
                    pi += 1
                    for cc in range(2):
                        nc.tensor.matmul(
                            pq[:, 0:512],
                            lhsT=w_sb[cc][:, 128 * oc : 128 * (oc + 1)],
                            rhs=x_sb[b][cc][:, 512 * nc2 : 512 * (nc2 + 1)],
                            start=(cc == 0),
                            stop=(cc == 1),
                        )
                    nc.vector.tensor_copy(
                        out=dst[b][oc][:, 512 * nc2 : 512 * (nc2 + 1)],
                        in_=pq[:, 0:512],
                    )
        for nt in range(8):
            pv = proj_psum(pi)
            pi += 1
            for cc in range(2):
                nc.tensor.matmul(
                    pv[:, 0:C],
                    lhsT=x_sb[b][cc][:, 128 * nt : 128 * (nt + 1)],
                    rhs=wv_sb[cc][:],
                    start=(cc == 0),
                    stop=(cc == 1),
                )
            v33 = v_sb[b][nt][:].rearrange("p (h w) -> p h w", w=33)
            nc.vector.tensor_copy(
                out=v33[:, :, 0:32], in_=pv[:, 0:C].rearrange("p (h w) -> p h w", w=32)
            )
            nc.vector.memset(v33[:, :, 32:33], 1.0)

    # ---------- stage B: attention, 4 heads (one quad) at a time ----------
    # out-projection rhs: chunk 0 = heads 0..3, chunk 1 = heads 4..7.
    otn_sb = [[persist.tile([128, N], BF16, tag=f"otn{b}_{ch}", name=f"otn{b}_{ch}") for ch in range(2)] for b in range(B_LOC)]

    # Software pipeline: PV matmuls lag one strip behind ST in the PE queue
    # (a stalled PV never head-of-line blocks ready ST work), and the previous
    # iteration's normalization chain (which has DMA bounces between DVE ops)
    # is drip-fed where its inputs are already available.
    pending_pv = None
    norm_parts = []

    def _emit_pv(args):
        otden_, b_, kt_, se_, quad_, first, last = args
        for h2 in range(4):
            nc.tensor.matmul(
                otden_[:, 512 * h2 : 512 * (h2 + 1)],
                lhsT=v_sb[b_][kt_][:, 33 * (4 * quad_ + h2) : 33 * (4 * quad_ + h2) + 33],
                rhs=se_[:, 512 * h2 : 512 * (h2 + 1)],
                start=first,
                stop=last,
            )

    def _make_norm(otden_, quad_, b_, qi_):
        # Normalize O^T by the softmax denominators (row 32 of each bank).
        # Reciprocal is iterative (8 cyc/elem/lane): reshape the [1, 2048]
        # denominator row to [32, 64] via a DRAM bounce to use 32 lanes, then
        # bounce again to broadcast the reciprocals across 32 partitions.
        state = {}

        def part1():
            den_sb = small.tile([1, 2048], F32, tag="den_sb", name="den_sb_t")
            nc.vector.tensor_copy(out=den_sb[:], in_=otden_[32:33, :])
            den_dr = dram_pool.tile([1, 2048], F32, tag="den_dr", name="den_dr")
            nc.sync.dma_start(out=den_dr[:], in_=den_sb[:])
            rden32 = small.tile([32, 64], F32, tag="rden32", name="rden32_t")
            nc.sync.dma_start(
                out=rden32[:], in_=den_dr[:].rearrange("one (p j) -> (one p) j", j=64)
            )
            state["rden32"] = rden32

        def part2():
            rden32 = state["rden32"]
            nc.vector.reciprocal(out=rden32[:], in_=rden32[:])
            rden_dr = dram_pool.tile([1, 2048], F32, tag="rden_dr", name="rden_dr")
            nc.sync.dma_start(
                out=rden_dr[:].rearrange("one (p j) -> (one p) j", j=64), in_=rden32[:]
            )
            rdb = small.tile([32, 2048], F32, tag="rdb", name="rdb_t")
            nc.sync.dma_start(out=rdb[:], in_=rden_dr[0:1, :].to_broadcast([32, 2048]))
            state["rdb"] = rdb

        def part3():
            rdb = state["rdb"]
            for h2 in range(4):
                nc.vector.tensor_mul(
                    out=otn_sb[b_][quad_][32 * h2 : 32 * (h2 + 1), 512 * qi_ : 512 * (qi_ + 1)],
                    in0=otden_[0:32, 512 * h2 : 512 * (h2 + 1)],
                    in1=rdb[:, 512 * h2 : 512 * (h2 + 1)],
                )

        return [part1, part2, part3]

    eb3 = eb_sb[:].rearrange("p (h w) -> p h w", w=GW)
    for quad in range(2):
        for b in range(B_LOC):
            for qi in range(2):  # query chunk of 512
                # otden: bank h2 holds head (4*quad+h2): rows 0-31 = O^T,
                # row 32 = denominator.  One accumulation group per bank.
                otden = ps_ot.tile([33, 2048], F32, tag="otden", name="otden_ps")
                for kt in range(8):
                    st = ps_st.tile([128, 2048], F32, tag="st", name="st_ps")
                    for h2 in range(4):
                        nc.tensor.matmul(
                            st[:, 512 * h2 : 512 * (h2 + 1)],
                            lhsT=kT_sb[b][quad][32 * h2 : 32 * (h2 + 1), 128 * kt : 128 * (kt + 1)],
                            rhs=qT_sb[b][quad][32 * h2 : 32 * (h2 + 1), 512 * qi : 512 * (qi + 1)],
                            start=True,
                            stop=True,
                            tile_position=(32 * h2, 0),
                        )
                    # exp(S+B) = exp(S) * exp(B): one wide exp on ScalarE
                    # (PSUM->SBUF, bf16), one strided bf16 2x multiply on
                    # VectorE against the 4 heads' exp-of-bias slices.
                    se = stexp_pool.tile([128, 2048], BF16, tag="se", name="se_t")
                    nc.scalar.activation(out=se[:], in_=st[:], func=AF.Exp)
                    off = 896 - 128 * kt + 512 * qi
                    nc.vector.tensor_mul(
                        out=se[:].rearrange("p (h q) -> p h q", h=4),
                        in0=se[:].rearrange("p (h q) -> p h q", h=4),
                        in1=eb3[:, 4 * quad : 4 * quad + 4, off : off + 512],
                    )
                    if pending_pv is not None:
                        _emit_pv(pending_pv)
                    if kt in (0, 1, 2) and norm_parts:
                        norm_parts.pop(0)()
                    pending_pv = (otden, b, kt, se, quad, kt == 0, kt == 7)
                while norm_parts:
                    norm_parts.pop(0)()
                norm_parts = _make_norm(otden, quad, b, qi)
    _emit_pv(pending_pv)
    while norm_parts:
        norm_parts.pop(0)()

    # ---------- stage C: output projection (+ bias), already in [c, n] layout ----------
    for b in range(B_LOC):
        for ct in range(2):
            for q2 in range(2):
                po = proj_psum(pi)
                pi += 1
                for ch in range(2):
                    nc.tensor.matmul(
                        po[:, 0:512],
                        lhsT=wo_sb[ch][:, 128 * ct : 128 * (ct + 1)],
                        rhs=otn_sb[b][ch][:, 512 * q2 : 512 * (q2 + 1)],
                        start=(ch == 0),
                        stop=(ch == 1),
                    )
                ob = outp.tile([128, 512], F32, tag="ob", name="ob_t")
                nc.scalar.activation(
                    out=ob[:], in_=po[:, 0:512], func=AF.Identity, bias=bo_sb[ct][:], scale=1.0
                )
                nc.sync.dma_start(
                    out=out[b, 128 * ct : 128 * (ct + 1), 512 * q2 : 512 * (q2 + 1)],
                    in_=ob[:],
                )


def build():
    nc = bacc.Bacc("TRN2", target_bir_lowering=False, debug=False, num_devices=N_CORES)
    io = {
        "x": nc.dram_tensor("x", [B_LOC, C, N], BF16, kind="ExternalInput").ap(),
        "wqT": nc.dram_tensor("wqT", [C, C], BF16, kind="ExternalInput").ap(),
        "wkT": nc.dram_tensor("wkT", [C, C], BF16, kind="ExternalInput").ap(),
        "wvT": nc.dram_tensor("wvT", [C, C], BF16, kind="ExternalInput").ap(),
        "woT": nc.dram_tensor("woT", [C, C], BF16, kind="ExternalInput").ap(),
        "bo": nc.dram_tensor("bo", [C, 1], F32, kind="ExternalInput").ap(),
        "eb": nc.dram_tensor("eb", [HEADS, 128, GW], BF16, kind="ExternalInput").ap(),
        "out": nc.dram_tensor("out", [B_LOC, C, N], F32, kind="ExternalOutput").ap(),
    }
    with tile.TileContext(nc) as tc:
        _emit(tc, io)
    nc.compile()
    return nc


_CACHE: dict = {}


def _get_nc():
    if "nc" not in _CACHE:
        _CACHE["nc"] = build()
    return _CACHE["nc"]


def make_in_maps(x, Wq, Wk, Wv, Wo, bo, rel_bias, rel_idx=None):
    """Host-side sharding/layout prep. Returns per-core input maps."""
    import ml_dtypes

    bf16 = ml_dtypes.bfloat16
    x = np.asarray(x, np.float32)
    b, c, H, W = x.shape
    assert (b, c, H * W) == (B_LOC * N_CORES, C, N)
    xr = np.ascontiguousarray(x.reshape(b, c, N).astype(bf16))
    wqT = np.ascontiguousarray(np.asarray(Wq, np.float32).T.astype(bf16))
    wkT = np.ascontiguousarray((np.asarray(Wk, np.float32) * SCALE).T.astype(bf16))
    wvT = np.ascontiguousarray(np.asarray(Wv, np.float32).T.astype(bf16))
    woT = np.ascontiguousarray(np.asarray(Wo, np.float32).T.astype(bf16))
    bo2 = np.ascontiguousarray(np.asarray(bo, np.float32).reshape(C, 1))
    rb = np.asarray(rel_bias, np.float32)
    idx = G0 + np.arange(128)[:, None] - np.arange(GW)[None, :]
    ebmat = np.ascontiguousarray(np.exp(rb[:, idx]).astype(bf16))  # [8, 128, GW]
    shared = dict(wqT=wqT, wkT=wkT, wvT=wvT, woT=woT, bo=bo2, eb=ebmat)
    return [
        dict(x=np.ascontiguousarray(xr[B_LOC * i : B_LOC * (i + 1)]), **shared)
        for i in range(N_CORES)
    ]


def _install_ntff_hook_shim():
    """bass_utils fetches the axon NTFF hook via antenv.axon_hooks, which this
    image's antenv lacks; synthesize it from trn_agent_boot's ctypes hook."""
    import sys
    import types

    try:
        from antenv.axon_hooks import get_axon_ntff_profile_hook  # noqa: F401

        return
    except ImportError:
        pass
    hook = None
    try:
        from trn_agent_boot.trn_boot import _ntff_profile_via_ctypes

        hook = _ntff_profile_via_ctypes("/opt/axon/libaxon_pjrt.so")
    except Exception:
        pass
    mod = types.ModuleType("antenv.axon_hooks")
    state = {"hook": hook}
    mod.get_axon_ntff_profile_hook = lambda: state["hook"]
    mod.set_axon_ntff_profile_hook = lambda h: state.__setitem__("hook", h)
    sys.modules["antenv.axon_hooks"] = mod


def run(inputs: dict, trace: bool = False):
    """Run on the 8 cores; returns (full_output, BassKernelResults)."""
    if trace:
        _install_ntff_hook_shim()
    in_maps = make_in_maps(**inputs)
    nc = _get_nc()
    res = bass_utils.run_bass_kernel_spmd(
        nc, in_maps, core_ids=list(range(N_CORES)), trace=trace
    )
    outs = np.stack([res.results[i]["out"] for i in range(N_CORES)])
    out = outs.reshape(B_LOC * N_CORES, C, 32, 32)
    return out, res


def kernel(**inputs) -> np.ndarray:
    out, _ = run(inputs)
    return out
